# revision 24
# baseline (speedup 1.0000x reference)
"""GRU (hard-sigmoid gates, tanh candidate) Trainium2 kernel, 8 NeuronCores.

Strategy:
  - Data-parallel: batch 32 -> 4 per core. The T=512 recurrence is strictly
    sequential, and collectives have a ~5us floor, so each core runs its own
    batch shard's full recurrence locally (no cross-core traffic).
  - Everything lives transposed: h^T [U_part, B_free], mx^T [3U_part, T, B].
    The recurrent matmul uses the native recurrent_kernel [U, 3U] layout as
    the stationary operand (lhsT), streaming h^T [128, 4] as the moving
    operand -> output lands already transposed, elementwise ops use all 128
    partitions, and no per-step transposes are needed.
  - bf16 weights/h (fp32 matmul is 4 cycles/row; bf16 is 1), f32 PSUM.
  - hard_sigmoid folding: z/r columns of both weight matrices are pre-scaled
    by 0.2 on the host and mx for those columns gets bias' = 0.2*bias + 0.5,
    so z|r = clip(psum, 0, 1) directly.
  - The mx additive term is folded into PSUM by an identity matmul that
    initializes each accumulation group (start=True), so no DVE adds are on
    the critical path; clip and tanh read PSUM directly.
  - Blend h' = z*h + (1-z)*hh is two post-tanh DVE ops: a=z*h and w=1-z are
    precomputed in the hh-matmul shadow; then f=w*hh, h'=f+a.
  - h' is written directly into a persistent bf16 history buffer (slot s+1);
    the next step's matmuls read slot s. History bulk-DMAs to DRAM (bf16)
    every 64 steps; host upcasts to f32.
  - Outer For_i hardware loop (8 iters) x 64 python-unrolled steps with
    ping-pong prefetch of the staged mx blocks.
"""

import os
import sys
from contextlib import ExitStack

sys.path.insert(0, "/opt/trn_rl_repo")

import numpy as np
import ml_dtypes

import concourse.bass as bass
import concourse.tile as tile
from concourse import bacc, mybir
from concourse.bass_utils import run_bass_kernel_spmd
from concourse.masks import make_identity
from concourse.tile_autobufs import add_dep_helper


def _install_ntff_hook():
    """The container's antenv stub lacks axon_hooks; provide it so
    trace=True (used by test.py for profiling) works. No-op on failure."""
    import types

    try:
        import antenv
        if "antenv.axon_hooks" in sys.modules:
            return
        mod = types.ModuleType("antenv.axon_hooks")
        state = {"h": None}
        mod.set_axon_ntff_profile_hook = lambda h: state.__setitem__("h", h)
        mod.get_axon_ntff_profile_hook = lambda: state["h"]
        sys.modules["antenv.axon_hooks"] = mod
        antenv.axon_hooks = mod
        from trn_agent_boot.trn_boot import _ntff_profile_via_ctypes
        mod.set_axon_ntff_profile_hook(
            _ntff_profile_via_ctypes("/opt/axon/libaxon_pjrt.so")
        )
    except Exception:
        pass


_install_ntff_hook()


def _register_gru_dve_ops():
    """Register two fused custom DVE ops (documented extension point:
    dve_ops.OPS + opcode row map). Cuts the critical chain from two DVE
    ops (clip, mul) to one, and the z-path from three to two:
      CLIP01_MUL: out = clip(in0, 0, 1) * in1   (rh = clip(pr)*h, a = clip(pz)*h)
      ANT_GRU_W:  out = 1 - clip(in0, 0, 1)     (w  = 1 - z)
    uops_sha is computed at registration (golden-pin is self-consistent)."""
    from concourse import dve_ops as DV
    from concourse.dve_spec import (
        Spec, Src0, Src1, Zero, One, minn, maxx, lower, _has_src1,
    )
    from concourse.dve_uop import DveOpSpec

    if "ANT_GRU_CLIP01_MUL" in DV._SUB_OPCODE_FOR_NAME:
        by_name = {op.name: op for op in DV.OPS}
        return by_name["ANT_GRU_CLIP01_MUL"], by_name["ANT_GRU_W"]

    def make(name, spec):
        opcode = DV._CUSTOM_DVE_ROW_BASE + len(DV.OPS)
        shas = {}
        for ver in ("v3", "v4"):
            s = DveOpSpec(name=name, opcode=opcode,
                          uops=lower(spec, ver=ver), rd1_en=_has_src1(spec))
            shas[ver] = s.sha(ver)
        op = DV.DveOp(name, spec, subdim=False, uops_sha=shas)
        DV.OPS.append(op)
        DV._SUB_OPCODE_FOR_NAME[name] = opcode
        return op

    clip_mul = make("ANT_GRU_CLIP01_MUL", Spec(
        body=minn(maxx(Src0, Zero), One) * Src1,
        reference=lambda in0, in1, s0, s1, imm2: (
            np.clip(in0, 0.0, 1.0) * in1),
    ))
    one_minus = make("ANT_GRU_W", Spec(
        body=One - minn(maxx(Src0, Zero), One),
        reference=lambda in0, s0, s1, imm2: 1.0 - np.clip(in0, 0.0, 1.0),
    ))
    return clip_mul, one_minus


CLIP01_MUL, GRU_W = _register_gru_dve_ops()

B, T, D, U = 32, 512, 512, 512
NCORES = 8
BL = B // NCORES          # 4 batches per core
KC = D // 128             # 4 contraction chunks (input proj)
UC = U // 128             # 4 contraction chunks (recurrent)
M_ALL = 3 * U // 128      # 12 output column chunks
SBLK = 64                 # steps per staged mx block
BODY = 2 * SBLK           # steps per For_i body (ping-pong A/B)

BF16 = mybir.dt.bfloat16
F32 = mybir.dt.float32
F8 = mybir.dt.float8e4
Alu = mybir.AluOpType
Act = mybir.ActivationFunctionType
ET = mybir.EngineType

_CACHE = {}
LAST_RESULT = None


def _build(T=T):
    nc = bacc.Bacc()
    xT = nc.declare_dram_parameter("xT", [D, BL * T], BF16, isOutput=False)
    wk = nc.declare_dram_parameter("wk", [D, 3 * U], BF16, isOutput=False)
    # recurrent weights split by gate: z stays bf16 (z scales h directly, most
    # error-sensitive); r and hh go fp8-e4m3 (errors squashed by clip/tanh) --
    # fp8 FWL halves the LDWEIGHTS cadence on the two critical-chain bursts
    wrz = nc.declare_dram_parameter("wrz", [U, U], BF16, isOutput=False)
    wrr = nc.declare_dram_parameter("wrr", [U, U], BF16, isOutput=False)
    wrh = nc.declare_dram_parameter("wrh", [U, U], BF16, isOutput=False)
    bp = nc.declare_dram_parameter("bp", [3 * U], F32, isOutput=False)
    # out[u%128, u//128, t, b] (bf16; host upcasts)
    out = nc.declare_dram_parameter("out", [128, UC, T, BL], BF16, isOutput=True)

    with tile.TileContext(nc) as tc, ExitStack() as ctx:
        consts = ctx.enter_context(tc.tile_pool(name="consts", bufs=1))
        psum_p = ctx.enter_context(tc.tile_pool(name="psum", bufs=2, space="PSUM"))
        psum_1 = ctx.enter_context(tc.tile_pool(name="psum1", bufs=1, space="PSUM"))
        work = ctx.enter_context(tc.tile_pool(name="work", bufs=2))

        wk_sb = consts.tile([128, KC, 3 * U], BF16)
        nc.sync.dma_start(out=wk_sb, in_=wk.rearrange("(c p) n -> p c n", p=128))
        wrz_sb = consts.tile([128, UC, U], BF16)
        nc.sync.dma_start(out=wrz_sb, in_=wrz.rearrange("(c p) n -> p c n", p=128))
        wrr_sb = consts.tile([128, UC, U], BF16)
        nc.sync.dma_start(out=wrr_sb, in_=wrr.rearrange("(c p) n -> p c n", p=128))
        wrh_sb = consts.tile([128, UC, U], BF16)
        nc.sync.dma_start(out=wrh_sb, in_=wrh.rearrange("(c p) n -> p c n", p=128))
        bp_sb = consts.tile([128, M_ALL], F32)
        nc.sync.dma_start(out=bp_sb, in_=bp.rearrange("(m p) -> p m", p=128))
        # chunked xT load so phase-1's first matmuls start after the first
        # d-chunk arrives instead of the whole tensor
        xT_sb = consts.tile([128, KC, BL * T], BF16)
        xT_r = xT.rearrange("(c p) n -> p c n", p=128)
        for d in range(KC):
            nc.sync.dma_start(out=xT_sb[:, d, :], in_=xT_r[:, d, :])
        ident = consts.tile([128, 128], BF16)
        make_identity(nc, ident)

        # mx^T [n%128, n//128, t, b] bf16, padded by BODY junk steps so the
        # ping-pong prefetch can always read a full block
        mx_sb = consts.tile([128, M_ALL, T + BODY, BL], BF16)
        nc.vector.memset(mx_sb[:, :, T:, :], 0.0)

        # ---- phase 1: mx^T = kernel^T @ x^T (+ bias', x0.2 pre-folded) ----
        # t-block-major so the first recurrence block's mx is ready after
        # 1/4 of phase1 (the rest overlaps the recurrence).
        xT_bt = xT_sb.rearrange("p c (b t) -> p c b t", b=BL)
        TB = T // 128
        for tb in range(TB):
            for m in range(M_ALL):
                ps = psum_p.tile([128, BL * 128], F32, tag="p1")
                for d in range(KC):
                    nc.tensor.matmul(
                        ps,
                        lhsT=wk_sb[:, d, m * 128:(m + 1) * 128],
                        rhs=xT_bt[:, d, :, tb * 128:(tb + 1) * 128],
                        start=(d == 0),
                        stop=(d == KC - 1),
                    )
                # psum free order is (b, t); reorder the mx view to match
                nc.scalar.activation(
                    out=mx_sb[:, m, tb * 128:(tb + 1) * 128, :].rearrange(
                        "p t b -> p b t"),
                    in_=ps, func=Act.Identity,
                    bias=bp_sb[:, m:m + 1],
                )

        # ---- phase 2: recurrence ----
        # persistent bf16 history: step s reads slot s, writes slot s+1;
        # the last step wraps to slot 0 (becomes next body's h_in) so no
        # carry copy is needed.
        hist = consts.tile([128, UC, BODY, BL], BF16)
        nc.vector.memset(hist[:, :, 0:1, :], 0.0)
        stgA = consts.tile([128, M_ALL, SBLK, BL], BF16)
        stgB = consts.tile([128, M_ALL, SBLK, BL], BF16)
        nc.sync.dma_start(out=stgA, in_=mx_sb[:, :, 0:SBLK, :])

        def step(stg, s, slot):
            out_slot = (slot + 1) % BODY
            h_in = hist[:, :, slot, :]                    # [128, UC, BL] bf16
            h_in4 = hist[:, :, slot:slot + 1, :]          # [128, UC, 1, BL]
            # Wide identity-MMs: init each gate-group psum from the staged mx
            # block in ONE LDW+MM pair each (N=16/16/8/8) instead of twelve
            # N=4 pairs. Separate banks per group so DVE/ACT reads never race
            # concurrent PE writes.
            pr = psum_1.tile([128, 4, 1, BL], F32, tag="pr")
            nc.tensor.matmul(
                pr[:, :, 0, :], lhsT=ident, rhs=stg[:, 4:8, s, :],
                start=True, stop=False, skip_group_check=True,
            )
            pz = psum_1.tile([128, 4, 1, BL], F32, tag="pz")
            nc.tensor.matmul(
                pz[:, :, 0, :], lhsT=ident, rhs=stg[:, 0:4, s, :],
                start=True, stop=False, skip_group_check=True,
            )
            # r-gate weight MMs first, k-outer so the k=0,1 MMs only need the
            # first half of the blended h (chunked handoff from prev step)
            r_last = None
            for k in range(UC):
                for m in range(4):
                    r_last = nc.tensor.matmul(
                        pr[:, m, 0, :],
                        lhsT=wrr_sb[:, k, m * 128:(m + 1) * 128],
                        rhs=h_in[:, k, :],
                        start=False,
                        stop=(k == UC - 1 and m == 3),
                        skip_group_check=True,
                    )
            # rh = clip(psum_r, 0, 1) * h in ONE fused DVE op (unblocks the
            # hh matmuls one op earlier on the critical chain)
            rh = work.tile([128, UC, 1, BL], BF16, tag="rh")
            rh_i = nc.vector._custom_dve(
                CLIP01_MUL, out=rh[:, :, 0, :], in0=pr[:, :, 0, :], in1=h_in)
            z_last = None
            for k in range(UC):
                for m in range(4):
                    zi = nc.tensor.matmul(
                        pz[:, m, 0, :],
                        lhsT=wrz_sb[:, k, m * 128:(m + 1) * 128],
                        rhs=h_in[:, k, :],
                        start=False,
                        stop=(k == UC - 1 and m == 3),
                        skip_group_check=True,
                    )
                    if k == 0 and m == 0:
                        # same-engine ordering only (no semaphore): keep the
                        # whole z-burst AFTER the r-burst on the PE so clip_r
                        # fires at r-end, with z filling the clip_r/rh window
                        add_dep_helper(zi.ins, r_last.ins, sync=False,
                                       reason="z-burst after r-burst on PE")
                    z_last = zi
            # z-path off the critical chain, fused: w = 1 - clip(pz,0,1),
            # a = clip(pz,0,1) * h (z never materialized)
            w_t = work.tile([128, 4, 1, BL], BF16, tag="wt")
            w_i = nc.vector._custom_dve(
                GRU_W, out=w_t[:, :, 0, :], in0=pz[:, :, 0, :])
            add_dep_helper(w_i.ins, rh_i.ins, sync=False,
                           reason="DVE critical chain first")
            a_t = work.tile([128, 4, 1, BL], BF16, tag="at")
            nc.vector._custom_dve(
                CLIP01_MUL, out=a_t[:, :, 0, :], in0=pz[:, :, 0, :], in1=h_in)
            # hh psum banks (bufs=1): WAR dep on the previous step's tanh
            # resolves during this step's r/z bursts, so no PE stall
            phA = psum_p.tile([128, 2, 1, BL], F32, tag="phA")
            ihA = nc.tensor.matmul(
                phA[:, :, 0, :], lhsT=ident, rhs=stg[:, 8:10, s, :],
                start=True, stop=False, skip_group_check=True,
            )
            add_dep_helper(ihA.ins, z_last.ins, sync=False,
                           reason="hh inits after z-burst on PE")
            phB = psum_p.tile([128, 2, 1, BL], F32, tag="phB")
            nc.tensor.matmul(
                phB[:, :, 0, :], lhsT=ident, rhs=stg[:, 10:12, s, :],
                start=True, stop=False, skip_group_check=True,
            )
            # hh pre-activation: psum = mx_h' + rh @ W_h; m-halves with the
            # tanh/blend for each half emitted right after its 8 MMs so each
            # half's chain starts as soon as its psum is complete
            hA_last = None
            for m in range(2):
                for k in range(UC):
                    hA_last = nc.tensor.matmul(
                        phA[:, m, 0, :],
                        lhsT=wrh_sb[:, k, m * 128:(m + 1) * 128],
                        rhs=rh[:, k, 0, :],
                        start=False,
                        stop=(m == 1 and k == UC - 1),
                        skip_group_check=True,
                    )
            # hh = tanh(psum); h' = (1-z)*hh + z*h -> hist out_slot (A half)
            hh_A = work.tile([128, 2, 1, BL], BF16, tag="hhA2")
            nc.scalar.activation(out=hh_A, in_=phA, func=Act.Tanh)
            f_A = work.tile([128, 2, 1, BL], BF16, tag="ftA")
            nc.vector.tensor_mul(f_A, w_t[:, 0:2, :, :], hh_A)
            nc.vector.tensor_add(hist[:, 0:2, out_slot:out_slot + 1, :],
                                 f_A, a_t[:, 0:2, :, :])
            for m in range(2, 4):
                for k in range(UC):
                    hB = nc.tensor.matmul(
                        phB[:, m - 2, 0, :],
                        lhsT=wrh_sb[:, k, m * 128:(m + 1) * 128],
                        rhs=rh[:, k, 0, :],
                        start=False,
                        stop=(m == 3 and k == UC - 1),
                        skip_group_check=True,
                    )
                    if m == 2 and k == 0:
                        # keep the B-half strictly after the A-half on the PE
                        # so tanh_A fires at the half-burst boundary
                        add_dep_helper(hB.ins, hA_last.ins, sync=False,
                                       reason="hh B-half after A-half on PE")
            hh_B = work.tile([128, 2, 1, BL], BF16, tag="hhB2")
            nc.scalar.activation(out=hh_B, in_=phB, func=Act.Tanh)
            f_B = work.tile([128, 2, 1, BL], BF16, tag="ftB")
            nc.vector.tensor_mul(f_B, w_t[:, 2:4, :, :], hh_B)
            nc.vector.tensor_add(hist[:, 2:4, out_slot:out_slot + 1, :],
                                 f_B, a_t[:, 2:4, :, :])

        with tc.For_i(0, T, BODY, staggered_reset=True,
                      hint_engines=(ET.PE, ET.DVE, ET.Activation,
                                    ET.SP, ET.Pool)) as i:
            nc.sync.dma_start(out=stgB,
                              in_=mx_sb[:, :, bass.ds(i + SBLK, SBLK), :])
            for s in range(SBLK):
                step(stgA, s, s)
            nc.sync.dma_start(out=stgA,
                              in_=mx_sb[:, :, bass.ds(i + BODY, SBLK), :])
            for s in range(SBLK):
                step(stgB, s, SBLK + s)
            nc.sync.dma_start(out=out[:, :, bass.ds(i, BODY - 1), :],
                              in_=hist[:, :, 1:BODY, :])
            nc.sync.dma_start(out=out[:, :, bass.ds(i + BODY - 1, 1), :],
                              in_=hist[:, :, 0:1, :])
    return nc


def _graph():
    if "nc" not in _CACHE:
        nc = _build()
        if not nc.is_finalized():
            nc.finalize()
        _CACHE["nc"] = nc
    return _CACHE["nc"]


def kernel(x, kernel, recurrent_kernel, bias):
    global LAST_RESULT
    x = np.asarray(x, dtype=np.float32)
    wk_f = np.asarray(kernel, dtype=np.float32)
    wr_f = np.asarray(recurrent_kernel, dtype=np.float32)
    b_f = np.asarray(bias, dtype=np.float32)

    # fold hard_sigmoid affine (0.2*x + 0.5) into the z|r weight columns/bias
    scale = np.ones((3 * U,), np.float32)
    scale[: 2 * U] = 0.2
    wk_h = (wk_f * scale).astype(ml_dtypes.bfloat16)
    wr_s = wr_f * scale
    wrz_h = wr_s[:, :U].astype(ml_dtypes.bfloat16)
    wrr_h = wr_s[:, U:2 * U].astype(ml_dtypes.bfloat16)
    wrh_h = wr_s[:, 2 * U:].astype(ml_dtypes.bfloat16)
    bp_h = np.where(np.arange(3 * U) < 2 * U, 0.2 * b_f + 0.5, b_f).astype(np.float32)

    in_maps = []
    for c in range(NCORES):
        xs = x[c * BL:(c + 1) * BL]                       # [BL, T, D]
        xTc = np.ascontiguousarray(
            xs.transpose(2, 0, 1).reshape(D, BL * T)
        ).astype(ml_dtypes.bfloat16)
        in_maps.append({"xT": xTc, "wk": wk_h, "wrz": wrz_h,
                        "wrr": wrr_h, "wrh": wrh_h, "bp": bp_h})

    res = run_bass_kernel_spmd(
        _graph(), in_maps, core_ids=list(range(NCORES)),
        trace=bool(os.environ.get("GRU_TRACE")),
    )
    LAST_RESULT = res

    outs = []
    for c in range(NCORES):
        arr = np.asarray(res.results[c]["out"]).astype(np.float32)
        outs.append(np.transpose(arr, (3, 2, 1, 0)).reshape(BL, T, U))
    return np.concatenate(outs, axis=0)



# revision 26
# speedup vs baseline: 1.2067x; 1.2067x over previous
"""GRU (hard-sigmoid gates, tanh candidate) Trainium2 kernel, 8 NeuronCores.

Strategy:
  - Data-parallel: batch 32 -> 4 per core. The T=512 recurrence is strictly
    sequential, and collectives have a ~5us floor, so each core runs its own
    batch shard's full recurrence locally (no cross-core traffic).
  - Everything lives transposed: h^T [U_part, B_free], mx^T [3U_part, T, B].
    The recurrent matmul uses the native recurrent_kernel [U, 3U] layout as
    the stationary operand (lhsT), streaming h^T [128, 4] as the moving
    operand -> output lands already transposed, elementwise ops use all 128
    partitions, and no per-step transposes are needed.
  - bf16 weights/h (fp32 matmul is 4 cycles/row; bf16 is 1), f32 PSUM.
  - hard_sigmoid folding: z/r columns of both weight matrices are pre-scaled
    by 0.2 on the host and mx for those columns gets bias' = 0.2*bias + 0.5,
    so z|r = clip(psum, 0, 1) directly.
  - The mx additive term is folded into PSUM by an identity matmul that
    initializes each accumulation group (start=True), so no DVE adds are on
    the critical path; clip and tanh read PSUM directly.
  - Blend h' = z*h + (1-z)*hh is two post-tanh DVE ops: a=z*h and w=1-z are
    precomputed in the hh-matmul shadow; then f=w*hh, h'=f+a.
  - h' is written directly into a persistent bf16 history buffer (slot s+1);
    the next step's matmuls read slot s. History bulk-DMAs to DRAM (bf16)
    every 64 steps; host upcasts to f32.
  - Outer For_i hardware loop (8 iters) x 64 python-unrolled steps with
    ping-pong prefetch of the staged mx blocks.
"""

import os
import sys
from contextlib import ExitStack

sys.path.insert(0, "/opt/trn_rl_repo")

import numpy as np
import ml_dtypes

import concourse.bass as bass
import concourse.tile as tile
from concourse import bacc, mybir
from concourse.bass_utils import run_bass_kernel_spmd
from concourse.masks import make_identity
from concourse.tile_autobufs import add_dep_helper


def _install_ntff_hook():
    """The container's antenv stub lacks axon_hooks; provide it so
    trace=True (used by test.py for profiling) works. No-op on failure."""
    import types

    try:
        import antenv
        if "antenv.axon_hooks" in sys.modules:
            return
        mod = types.ModuleType("antenv.axon_hooks")
        state = {"h": None}
        mod.set_axon_ntff_profile_hook = lambda h: state.__setitem__("h", h)
        mod.get_axon_ntff_profile_hook = lambda: state["h"]
        sys.modules["antenv.axon_hooks"] = mod
        antenv.axon_hooks = mod
        from trn_agent_boot.trn_boot import _ntff_profile_via_ctypes
        mod.set_axon_ntff_profile_hook(
            _ntff_profile_via_ctypes("/opt/axon/libaxon_pjrt.so")
        )
    except Exception:
        pass


_install_ntff_hook()


def _register_gru_dve_ops():
    """Register two fused custom DVE ops (documented extension point:
    dve_ops.OPS + opcode row map). Cuts the critical chain from two DVE
    ops (clip, mul) to one, and the z-path from three to two:
      CLIP01_MUL: out = clip(in0, 0, 1) * in1   (rh = clip(pr)*h, a = clip(pz)*h)
      ANT_GRU_W:  out = 1 - clip(in0, 0, 1)     (w  = 1 - z)
    uops_sha is computed at registration (golden-pin is self-consistent)."""
    from concourse import dve_ops as DV
    from concourse.dve_spec import (
        Spec, Src0, Src1, Zero, One, minn, maxx, lower, _has_src1,
    )
    from concourse.dve_uop import DveOpSpec

    if "ANT_GRU_CLIP01_MUL" in DV._SUB_OPCODE_FOR_NAME:
        by_name = {op.name: op for op in DV.OPS}
        return by_name["ANT_GRU_CLIP01_MUL"], by_name["ANT_GRU_W"]

    def make(name, spec):
        opcode = DV._CUSTOM_DVE_ROW_BASE + len(DV.OPS)
        shas = {}
        for ver in ("v3", "v4"):
            s = DveOpSpec(name=name, opcode=opcode,
                          uops=lower(spec, ver=ver), rd1_en=_has_src1(spec))
            shas[ver] = s.sha(ver)
        op = DV.DveOp(name, spec, subdim=False, uops_sha=shas)
        DV.OPS.append(op)
        DV._SUB_OPCODE_FOR_NAME[name] = opcode
        return op

    clip_mul = make("ANT_GRU_CLIP01_MUL", Spec(
        body=minn(maxx(Src0, Zero), One) * Src1,
        reference=lambda in0, in1, s0, s1, imm2: (
            np.clip(in0, 0.0, 1.0) * in1),
    ))
    one_minus = make("ANT_GRU_W", Spec(
        body=One - minn(maxx(Src0, Zero), One),
        reference=lambda in0, s0, s1, imm2: 1.0 - np.clip(in0, 0.0, 1.0),
    ))
    return clip_mul, one_minus


CLIP01_MUL, GRU_W = _register_gru_dve_ops()

B, T, D, U = 32, 512, 512, 512
NCORES = 8
BL = B // NCORES          # 4 batches per core
KC = D // 128             # 4 contraction chunks (input proj)
UC = U // 128             # 4 contraction chunks (recurrent)
M_ALL = 3 * U // 128      # 12 output column chunks
SBLK = 64                 # steps per staged mx block
BODY = 2 * SBLK           # steps per For_i body (ping-pong A/B)

BF16 = mybir.dt.bfloat16
F32 = mybir.dt.float32
F8 = mybir.dt.float8e4
Alu = mybir.AluOpType
Act = mybir.ActivationFunctionType
ET = mybir.EngineType

_CACHE = {}
LAST_RESULT = None


def _build(T=T):
    nc = bacc.Bacc()
    xT = nc.declare_dram_parameter("xT", [D, BL * T], BF16, isOutput=False)
    wk = nc.declare_dram_parameter("wk", [D, 3 * U], BF16, isOutput=False)
    # recurrent weights split by gate: z stays bf16 (z scales h directly, most
    # error-sensitive); r and hh go fp8-e4m3 (errors squashed by clip/tanh) --
    # fp8 FWL halves the LDWEIGHTS cadence on the two critical-chain bursts
    wrz = nc.declare_dram_parameter("wrz", [U, U], BF16, isOutput=False)
    wrr = nc.declare_dram_parameter("wrr", [U, U], BF16, isOutput=False)
    wrh = nc.declare_dram_parameter("wrh", [U, U], BF16, isOutput=False)
    bp = nc.declare_dram_parameter("bp", [3 * U], F32, isOutput=False)
    # out[u%128, u//128, t, b] (bf16; host upcasts)
    out = nc.declare_dram_parameter("out", [128, UC, T, BL], BF16, isOutput=True)

    with tile.TileContext(nc) as tc, ExitStack() as ctx:
        consts = ctx.enter_context(tc.tile_pool(name="consts", bufs=1))
        psum_p = ctx.enter_context(tc.tile_pool(name="psum", bufs=2, space="PSUM"))
        psum_1 = ctx.enter_context(tc.tile_pool(name="psum1", bufs=1, space="PSUM"))
        work = ctx.enter_context(tc.tile_pool(name="work", bufs=2))

        wk_sb = consts.tile([128, KC, 3 * U], BF16)
        nc.sync.dma_start(out=wk_sb, in_=wk.rearrange("(c p) n -> p c n", p=128))
        wrz_sb = consts.tile([128, UC, U], BF16)
        nc.sync.dma_start(out=wrz_sb, in_=wrz.rearrange("(c p) n -> p c n", p=128))
        wrr_sb = consts.tile([128, UC, U], BF16)
        nc.sync.dma_start(out=wrr_sb, in_=wrr.rearrange("(c p) n -> p c n", p=128))
        wrh_sb = consts.tile([128, UC, U], BF16)
        nc.sync.dma_start(out=wrh_sb, in_=wrh.rearrange("(c p) n -> p c n", p=128))
        bp_sb = consts.tile([128, M_ALL], F32)
        nc.sync.dma_start(out=bp_sb, in_=bp.rearrange("(m p) -> p m", p=128))
        # chunked xT load so phase-1's first matmuls start after the first
        # d-chunk arrives instead of the whole tensor
        xT_sb = consts.tile([128, KC, BL * T], BF16)
        xT_r = xT.rearrange("(c p) n -> p c n", p=128)
        for d in range(KC):
            nc.sync.dma_start(out=xT_sb[:, d, :], in_=xT_r[:, d, :])
        ident = consts.tile([128, 128], BF16)
        make_identity(nc, ident)

        # mx^T [n%128, n//128, t, b] bf16, padded by BODY junk steps so the
        # ping-pong prefetch can always read a full block
        mx_sb = consts.tile([128, M_ALL, T + BODY, BL], BF16)
        nc.vector.memset(mx_sb[:, :, T:, :], 0.0)

        # ---- phase 1: mx^T = kernel^T @ x^T (+ bias', x0.2 pre-folded) ----
        # t-block-major so the first recurrence block's mx is ready after
        # 1/4 of phase1 (the rest overlaps the recurrence).
        xT_bt = xT_sb.rearrange("p c (b t) -> p c b t", b=BL)
        TB = T // 128
        for tb in range(TB):
            for m in range(M_ALL):
                ps = psum_p.tile([128, BL * 128], F32, tag="p1")
                for d in range(KC):
                    nc.tensor.matmul(
                        ps,
                        lhsT=wk_sb[:, d, m * 128:(m + 1) * 128],
                        rhs=xT_bt[:, d, :, tb * 128:(tb + 1) * 128],
                        start=(d == 0),
                        stop=(d == KC - 1),
                    )
                # psum free order is (b, t); reorder the mx view to match
                nc.scalar.activation(
                    out=mx_sb[:, m, tb * 128:(tb + 1) * 128, :].rearrange(
                        "p t b -> p b t"),
                    in_=ps, func=Act.Identity,
                    bias=bp_sb[:, m:m + 1],
                )

        # ---- phase 2: recurrence ----
        # persistent bf16 history: step s reads slot s, writes slot s+1;
        # the last step wraps to slot 0 (becomes next body's h_in) so no
        # carry copy is needed.
        hist = consts.tile([128, UC, BODY, BL], BF16)
        nc.vector.memset(hist[:, :, 0:1, :], 0.0)
        stgA = consts.tile([128, M_ALL, SBLK, BL], BF16)
        stgB = consts.tile([128, M_ALL, SBLK, BL], BF16)
        nc.sync.dma_start(out=stgA, in_=mx_sb[:, :, 0:SBLK, :])

        def make_ids(stg, s):
            """Wide identity-MMs: init each gate-group psum from the staged mx
            block in ONE LDW+MM pair each (N=16/16/8/8). Separate banks per
            group so DVE/ACT reads never race concurrent PE writes. Called at
            the TAIL of the previous step so these fill the tanh/blend stall
            and keep the PE LDW pipeline warm."""
            pr = psum_1.tile([128, 4, 1, BL], F32, tag="pr")
            nc.tensor.matmul(
                pr[:, :, 0, :], lhsT=ident, rhs=stg[:, 4:8, s, :],
                start=True, stop=False, skip_group_check=True,
            )
            pz = psum_1.tile([128, 4, 1, BL], F32, tag="pz")
            nc.tensor.matmul(
                pz[:, :, 0, :], lhsT=ident, rhs=stg[:, 0:4, s, :],
                start=True, stop=False, skip_group_check=True,
            )
            phA = psum_p.tile([128, 2, 1, BL], F32, tag="phA")
            nc.tensor.matmul(
                phA[:, :, 0, :], lhsT=ident, rhs=stg[:, 8:10, s, :],
                start=True, stop=False, skip_group_check=True,
            )
            phB = psum_p.tile([128, 2, 1, BL], F32, tag="phB")
            nc.tensor.matmul(
                phB[:, :, 0, :], lhsT=ident, rhs=stg[:, 10:12, s, :],
                start=True, stop=False, skip_group_check=True,
            )
            return pr, pz, phA, phB

        def step(stg, s, slot, pre, nxt):
            out_slot = (slot + 1) % BODY
            h_in = hist[:, :, slot, :]                    # [128, UC, BL] bf16
            h_in4 = hist[:, :, slot:slot + 1, :]          # [128, UC, 1, BL]
            pr, pz, phA, phB = pre if pre is not None else make_ids(stg, s)
            # r-gate weight MMs first, k-outer so the k=0,1 MMs only need the
            # first half of the blended h (chunked handoff from prev step)
            r_last = None
            for k in range(UC):
                for m in range(4):
                    r_last = nc.tensor.matmul(
                        pr[:, m, 0, :],
                        lhsT=wrr_sb[:, k, m * 128:(m + 1) * 128],
                        rhs=h_in[:, k, :],
                        start=False,
                        stop=(k == UC - 1 and m == 3),
                        skip_group_check=True,
                    )
            # rh = clip(psum_r, 0, 1) * h in ONE fused DVE op (unblocks the
            # hh matmuls one op earlier on the critical chain)
            rh = work.tile([128, UC, 1, BL], BF16, tag="rh")
            rh_i = nc.vector._custom_dve(
                CLIP01_MUL, out=rh[:, :, 0, :], in0=pr[:, :, 0, :], in1=h_in)
            z_last = None
            for k in range(UC):
                for m in range(4):
                    zi = nc.tensor.matmul(
                        pz[:, m, 0, :],
                        lhsT=wrz_sb[:, k, m * 128:(m + 1) * 128],
                        rhs=h_in[:, k, :],
                        start=False,
                        stop=(k == UC - 1 and m == 3),
                        skip_group_check=True,
                    )
                    if k == 0 and m == 0:
                        # same-engine ordering only (no semaphore): keep the
                        # whole z-burst AFTER the r-burst on the PE so clip_r
                        # fires at r-end, with z filling the clip_r/rh window
                        add_dep_helper(zi.ins, r_last.ins, sync=False,
                                       reason="z-burst after r-burst on PE")
                    z_last = zi
            # z-path off the critical chain, fused: w = 1 - clip(pz,0,1),
            # a = clip(pz,0,1) * h (z never materialized)
            w_t = work.tile([128, 4, 1, BL], BF16, tag="wt")
            w_i = nc.vector._custom_dve(
                GRU_W, out=w_t[:, :, 0, :], in0=pz[:, :, 0, :])
            add_dep_helper(w_i.ins, rh_i.ins, sync=False,
                           reason="DVE critical chain first")
            a_t = work.tile([128, 4, 1, BL], BF16, tag="at")
            nc.vector._custom_dve(
                CLIP01_MUL, out=a_t[:, :, 0, :], in0=pz[:, :, 0, :], in1=h_in)
            # hh pre-activation: psum = mx_h' + rh @ W_h; m-halves with the
            # tanh/blend for each half emitted right after its 8 MMs so each
            # half's chain starts as soon as its psum is complete
            hA_last = None
            for m in range(2):
                for k in range(UC):
                    hA_last = nc.tensor.matmul(
                        phA[:, m, 0, :],
                        lhsT=wrh_sb[:, k, m * 128:(m + 1) * 128],
                        rhs=rh[:, k, 0, :],
                        start=False,
                        stop=(m == 1 and k == UC - 1),
                        skip_group_check=True,
                    )
            # hh = tanh(psum); h' = (1-z)*hh + z*h -> hist out_slot (A half)
            hh_A = work.tile([128, 2, 1, BL], BF16, tag="hhA2")
            nc.scalar.activation(out=hh_A, in_=phA, func=Act.Tanh)
            f_A = work.tile([128, 2, 1, BL], BF16, tag="ftA")
            nc.vector.tensor_mul(f_A, w_t[:, 0:2, :, :], hh_A)
            nc.vector.tensor_add(hist[:, 0:2, out_slot:out_slot + 1, :],
                                 f_A, a_t[:, 0:2, :, :])
            for m in range(2, 4):
                for k in range(UC):
                    hB = nc.tensor.matmul(
                        phB[:, m - 2, 0, :],
                        lhsT=wrh_sb[:, k, m * 128:(m + 1) * 128],
                        rhs=rh[:, k, 0, :],
                        start=False,
                        stop=(m == 3 and k == UC - 1),
                        skip_group_check=True,
                    )

            hh_B = work.tile([128, 2, 1, BL], BF16, tag="hhB2")
            nc.scalar.activation(out=hh_B, in_=phB, func=Act.Tanh)
            f_B = work.tile([128, 2, 1, BL], BF16, tag="ftB")
            nc.vector.tensor_mul(f_B, w_t[:, 2:4, :, :], hh_B)
            nc.vector.tensor_add(hist[:, 2:4, out_slot:out_slot + 1, :],
                                 f_B, a_t[:, 2:4, :, :])
            return make_ids(*nxt) if nxt is not None else None

        with tc.For_i(0, T, BODY, staggered_reset=True,
                      hint_engines=(ET.PE, ET.DVE, ET.Activation,
                                    ET.SP, ET.Pool)) as i:
            nc.sync.dma_start(out=stgB,
                              in_=mx_sb[:, :, bass.ds(i + SBLK, SBLK), :])
            pre = None
            for s in range(SBLK):
                nxt = (stgA, s + 1) if s < SBLK - 1 else (stgB, 0)
                pre = step(stgA, s, s, pre, nxt)
            nc.sync.dma_start(out=stgA,
                              in_=mx_sb[:, :, bass.ds(i + BODY, SBLK), :])
            for s in range(SBLK):
                nxt = (stgB, s + 1) if s < SBLK - 1 else None
                pre = step(stgB, s, SBLK + s, pre, nxt)
            nc.sync.dma_start(out=out[:, :, bass.ds(i, BODY - 1), :],
                              in_=hist[:, :, 1:BODY, :])
            nc.sync.dma_start(out=out[:, :, bass.ds(i + BODY - 1, 1), :],
                              in_=hist[:, :, 0:1, :])
    return nc


def _graph():
    if "nc" not in _CACHE:
        nc = _build()
        if not nc.is_finalized():
            nc.finalize()
        _CACHE["nc"] = nc
    return _CACHE["nc"]


def kernel(x, kernel, recurrent_kernel, bias):
    global LAST_RESULT
    x = np.asarray(x, dtype=np.float32)
    wk_f = np.asarray(kernel, dtype=np.float32)
    wr_f = np.asarray(recurrent_kernel, dtype=np.float32)
    b_f = np.asarray(bias, dtype=np.float32)

    # fold hard_sigmoid affine (0.2*x + 0.5) into the z|r weight columns/bias
    scale = np.ones((3 * U,), np.float32)
    scale[: 2 * U] = 0.2
    wk_h = (wk_f * scale).astype(ml_dtypes.bfloat16)
    wr_s = wr_f * scale
    wrz_h = wr_s[:, :U].astype(ml_dtypes.bfloat16)
    wrr_h = wr_s[:, U:2 * U].astype(ml_dtypes.bfloat16)
    wrh_h = wr_s[:, 2 * U:].astype(ml_dtypes.bfloat16)
    bp_h = np.where(np.arange(3 * U) < 2 * U, 0.2 * b_f + 0.5, b_f).astype(np.float32)

    in_maps = []
    for c in range(NCORES):
        xs = x[c * BL:(c + 1) * BL]                       # [BL, T, D]
        xTc = np.ascontiguousarray(
            xs.transpose(2, 0, 1).reshape(D, BL * T)
        ).astype(ml_dtypes.bfloat16)
        in_maps.append({"xT": xTc, "wk": wk_h, "wrz": wrz_h,
                        "wrr": wrr_h, "wrh": wrh_h, "bp": bp_h})

    res = run_bass_kernel_spmd(
        _graph(), in_maps, core_ids=list(range(NCORES)),
        trace=bool(os.environ.get("GRU_TRACE")),
    )
    LAST_RESULT = res

    outs = []
    for c in range(NCORES):
        arr = np.asarray(res.results[c]["out"]).astype(np.float32)
        outs.append(np.transpose(arr, (3, 2, 1, 0)).reshape(BL, T, U))
    return np.concatenate(outs, axis=0)



# revision 27
# speedup vs baseline: 1.2227x; 1.0132x over previous
"""GRU (hard-sigmoid gates, tanh candidate) Trainium2 kernel, 8 NeuronCores.

Strategy:
  - Data-parallel: batch 32 -> 4 per core. The T=512 recurrence is strictly
    sequential, and collectives have a ~5us floor, so each core runs its own
    batch shard's full recurrence locally (no cross-core traffic).
  - Everything lives transposed: h^T [U_part, B_free], mx^T [3U_part, T, B].
    The recurrent matmul uses the native recurrent_kernel [U, 3U] layout as
    the stationary operand (lhsT), streaming h^T [128, 4] as the moving
    operand -> output lands already transposed, elementwise ops use all 128
    partitions, and no per-step transposes are needed.
  - bf16 weights/h (fp32 matmul is 4 cycles/row; bf16 is 1), f32 PSUM.
  - hard_sigmoid folding: z/r columns of both weight matrices are pre-scaled
    by 0.2 on the host and mx for those columns gets bias' = 0.2*bias + 0.5,
    so z|r = clip(psum, 0, 1) directly.
  - The mx additive term is folded into PSUM by an identity matmul that
    initializes each accumulation group (start=True), so no DVE adds are on
    the critical path; clip and tanh read PSUM directly.
  - Blend h' = z*h + (1-z)*hh is two post-tanh DVE ops: a=z*h and w=1-z are
    precomputed in the hh-matmul shadow; then f=w*hh, h'=f+a.
  - h' is written directly into a persistent bf16 history buffer (slot s+1);
    the next step's matmuls read slot s. History bulk-DMAs to DRAM (bf16)
    every 64 steps; host upcasts to f32.
  - Outer For_i hardware loop (8 iters) x 64 python-unrolled steps with
    ping-pong prefetch of the staged mx blocks.
"""

import os
import sys
from contextlib import ExitStack

sys.path.insert(0, "/opt/trn_rl_repo")

import numpy as np
import ml_dtypes

import concourse.bass as bass
import concourse.tile as tile
from concourse import bacc, mybir
from concourse.bass_utils import run_bass_kernel_spmd
from concourse.masks import make_identity
from concourse.tile_autobufs import add_dep_helper


def _install_ntff_hook():
    """The container's antenv stub lacks axon_hooks; provide it so
    trace=True (used by test.py for profiling) works. No-op on failure."""
    import types

    try:
        import antenv
        if "antenv.axon_hooks" in sys.modules:
            return
        mod = types.ModuleType("antenv.axon_hooks")
        state = {"h": None}
        mod.set_axon_ntff_profile_hook = lambda h: state.__setitem__("h", h)
        mod.get_axon_ntff_profile_hook = lambda: state["h"]
        sys.modules["antenv.axon_hooks"] = mod
        antenv.axon_hooks = mod
        from trn_agent_boot.trn_boot import _ntff_profile_via_ctypes
        mod.set_axon_ntff_profile_hook(
            _ntff_profile_via_ctypes("/opt/axon/libaxon_pjrt.so")
        )
    except Exception:
        pass


_install_ntff_hook()


def _register_gru_dve_ops():
    """Register two fused custom DVE ops (documented extension point:
    dve_ops.OPS + opcode row map). Cuts the critical chain from two DVE
    ops (clip, mul) to one, and the z-path from three to two:
      CLIP01_MUL: out = clip(in0, 0, 1) * in1   (rh = clip(pr)*h, a = clip(pz)*h)
      ANT_GRU_W:  out = 1 - clip(in0, 0, 1)     (w  = 1 - z)
    uops_sha is computed at registration (golden-pin is self-consistent)."""
    from concourse import dve_ops as DV
    from concourse.dve_spec import (
        Spec, Src0, Src1, Zero, One, minn, maxx, lower, _has_src1,
    )
    from concourse.dve_uop import DveOpSpec

    if "ANT_GRU_CLIP01_MUL" in DV._SUB_OPCODE_FOR_NAME:
        by_name = {op.name: op for op in DV.OPS}
        return by_name["ANT_GRU_CLIP01_MUL"], by_name["ANT_GRU_W"]

    def make(name, spec):
        opcode = DV._CUSTOM_DVE_ROW_BASE + len(DV.OPS)
        shas = {}
        for ver in ("v3", "v4"):
            s = DveOpSpec(name=name, opcode=opcode,
                          uops=lower(spec, ver=ver), rd1_en=_has_src1(spec))
            shas[ver] = s.sha(ver)
        op = DV.DveOp(name, spec, subdim=False, uops_sha=shas)
        DV.OPS.append(op)
        DV._SUB_OPCODE_FOR_NAME[name] = opcode
        return op

    clip_mul = make("ANT_GRU_CLIP01_MUL", Spec(
        body=minn(maxx(Src0, Zero), One) * Src1,
        reference=lambda in0, in1, s0, s1, imm2: (
            np.clip(in0, 0.0, 1.0) * in1),
    ))
    one_minus = make("ANT_GRU_W", Spec(
        body=One - minn(maxx(Src0, Zero), One),
        reference=lambda in0, s0, s1, imm2: 1.0 - np.clip(in0, 0.0, 1.0),
    ))
    return clip_mul, one_minus


CLIP01_MUL, GRU_W = _register_gru_dve_ops()

B, T, D, U = 32, 512, 512, 512
NCORES = 8
BL = B // NCORES          # 4 batches per core
KC = D // 128             # 4 contraction chunks (input proj)
UC = U // 128             # 4 contraction chunks (recurrent)
M_ALL = 3 * U // 128      # 12 output column chunks
SBLK = 128                # steps per staged mx block
BODY = 2 * SBLK           # steps per For_i body (ping-pong A/B)

BF16 = mybir.dt.bfloat16
F32 = mybir.dt.float32
F8 = mybir.dt.float8e4
Alu = mybir.AluOpType
Act = mybir.ActivationFunctionType
ET = mybir.EngineType

_CACHE = {}
LAST_RESULT = None


def _build(T=T):
    nc = bacc.Bacc()
    xT = nc.declare_dram_parameter("xT", [D, BL * T], BF16, isOutput=False)
    wk = nc.declare_dram_parameter("wk", [D, 3 * U], BF16, isOutput=False)
    # recurrent weights split by gate: z stays bf16 (z scales h directly, most
    # error-sensitive); r and hh go fp8-e4m3 (errors squashed by clip/tanh) --
    # fp8 FWL halves the LDWEIGHTS cadence on the two critical-chain bursts
    wrz = nc.declare_dram_parameter("wrz", [U, U], BF16, isOutput=False)
    wrr = nc.declare_dram_parameter("wrr", [U, U], BF16, isOutput=False)
    wrh = nc.declare_dram_parameter("wrh", [U, U], BF16, isOutput=False)
    bp = nc.declare_dram_parameter("bp", [3 * U], F32, isOutput=False)
    # out[u%128, u//128, t, b] (bf16; host upcasts)
    out = nc.declare_dram_parameter("out", [128, UC, T, BL], BF16, isOutput=True)

    with tile.TileContext(nc) as tc, ExitStack() as ctx:
        consts = ctx.enter_context(tc.tile_pool(name="consts", bufs=1))
        psum_p = ctx.enter_context(tc.tile_pool(name="psum", bufs=2, space="PSUM"))
        psum_1 = ctx.enter_context(tc.tile_pool(name="psum1", bufs=1, space="PSUM"))
        work = ctx.enter_context(tc.tile_pool(name="work", bufs=2))

        # phase-1 inputs first: chunked xT (first matmuls start after the
        # first d-chunk arrives), then wk per-column-block, then bias.
        # Recurrent weights are DMA'd after phase-1 emission -- they are not
        # needed until the recurrence starts ~60us later.
        xT_sb = consts.tile([128, KC, BL * T], BF16)
        xT_r = xT.rearrange("(c p) n -> p c n", p=128)
        wk_sb = consts.tile([128, KC, 3 * U], BF16)
        wk_r = wk.rearrange("(c p) n -> p c n", p=128)
        nc.sync.dma_start(out=xT_sb[:, 0, :], in_=xT_r[:, 0, :])
        nc.sync.dma_start(out=wk_sb[:, 0, :], in_=wk_r[:, 0, :])
        for d in range(1, KC):
            nc.sync.dma_start(out=xT_sb[:, d, :], in_=xT_r[:, d, :])
            nc.sync.dma_start(out=wk_sb[:, d, :], in_=wk_r[:, d, :])
        bp_sb = consts.tile([128, M_ALL], F32)
        nc.sync.dma_start(out=bp_sb, in_=bp.rearrange("(m p) -> p m", p=128))
        wrz_sb = consts.tile([128, UC, U], BF16)
        wrr_sb = consts.tile([128, UC, U], BF16)
        wrh_sb = consts.tile([128, UC, U], BF16)
        ident = consts.tile([128, 128], BF16)
        make_identity(nc, ident)

        # mx^T [n%128, n//128, t, b] bf16, padded by BODY junk steps so the
        # ping-pong prefetch can always read a full block
        mx_sb = consts.tile([128, M_ALL, T + BODY, BL], BF16)
        nc.vector.memset(mx_sb[:, :, T:, :], 0.0)

        # ---- phase 1: mx^T = kernel^T @ x^T (+ bias', x0.2 pre-folded) ----
        # t-block-major so the first recurrence block's mx is ready after
        # 1/4 of phase1 (the rest overlaps the recurrence).
        xT_bt = xT_sb.rearrange("p c (b t) -> p c b t", b=BL)
        TB = T // 128
        for tb in range(TB):
            for m in range(M_ALL):
                ps = psum_p.tile([128, BL * 128], F32, tag="p1")
                for d in range(KC):
                    nc.tensor.matmul(
                        ps,
                        lhsT=wk_sb[:, d, m * 128:(m + 1) * 128],
                        rhs=xT_bt[:, d, :, tb * 128:(tb + 1) * 128],
                        start=(d == 0),
                        stop=(d == KC - 1),
                    )
                # psum free order is (b, t); reorder the mx view to match
                nc.scalar.activation(
                    out=mx_sb[:, m, tb * 128:(tb + 1) * 128, :].rearrange(
                        "p t b -> p b t"),
                    in_=ps, func=Act.Identity,
                    bias=bp_sb[:, m:m + 1],
                )

        nc.sync.dma_start(out=wrz_sb, in_=wrz.rearrange("(c p) n -> p c n", p=128))
        nc.sync.dma_start(out=wrr_sb, in_=wrr.rearrange("(c p) n -> p c n", p=128))
        nc.sync.dma_start(out=wrh_sb, in_=wrh.rearrange("(c p) n -> p c n", p=128))

        # ---- phase 2: recurrence ----
        # persistent bf16 history: step s reads slot s, writes slot s+1;
        # the last step wraps to slot 0 (becomes next body's h_in) so no
        # carry copy is needed.
        hist = consts.tile([128, UC, BODY, BL], BF16)
        nc.vector.memset(hist[:, :, 0:1, :], 0.0)
        stgA = consts.tile([128, M_ALL, SBLK, BL], BF16)
        stgB = consts.tile([128, M_ALL, SBLK, BL], BF16)
        nc.sync.dma_start(out=stgA, in_=mx_sb[:, :, 0:SBLK, :])

        def make_ids(stg, s):
            """Wide identity-MMs: init each gate-group psum from the staged mx
            block in ONE LDW+MM pair each (N=16/16/8/8). Separate banks per
            group so DVE/ACT reads never race concurrent PE writes. Called at
            the TAIL of the previous step so these fill the tanh/blend stall
            and keep the PE LDW pipeline warm."""
            pr = psum_1.tile([128, 4, 1, BL], F32, tag="pr")
            nc.tensor.matmul(
                pr[:, :, 0, :], lhsT=ident, rhs=stg[:, 4:8, s, :],
                start=True, stop=False, skip_group_check=True,
            )
            pz = psum_1.tile([128, 4, 1, BL], F32, tag="pz")
            nc.tensor.matmul(
                pz[:, :, 0, :], lhsT=ident, rhs=stg[:, 0:4, s, :],
                start=True, stop=False, skip_group_check=True,
            )
            phA = psum_p.tile([128, 2, 1, BL], F32, tag="phA")
            nc.tensor.matmul(
                phA[:, :, 0, :], lhsT=ident, rhs=stg[:, 8:10, s, :],
                start=True, stop=False, skip_group_check=True,
            )
            phB = psum_p.tile([128, 2, 1, BL], F32, tag="phB")
            nc.tensor.matmul(
                phB[:, :, 0, :], lhsT=ident, rhs=stg[:, 10:12, s, :],
                start=True, stop=False, skip_group_check=True,
            )
            return pr, pz, phA, phB

        def step(stg, s, slot, pre, nxt):
            out_slot = (slot + 1) % BODY
            h_in = hist[:, :, slot, :]                    # [128, UC, BL] bf16
            h_in4 = hist[:, :, slot:slot + 1, :]          # [128, UC, 1, BL]
            pr, pz, phA, phB = pre if pre is not None else make_ids(stg, s)
            # r-gate weight MMs first, k-outer so the k=0,1 MMs only need the
            # first half of the blended h (chunked handoff from prev step)
            r_last = None
            for k in range(UC):
                for m in range(4):
                    r_last = nc.tensor.matmul(
                        pr[:, m, 0, :],
                        lhsT=wrr_sb[:, k, m * 128:(m + 1) * 128],
                        rhs=h_in[:, k, :],
                        start=False,
                        stop=(k == UC - 1 and m == 3),
                        skip_group_check=True,
                    )
            # rh = clip(psum_r, 0, 1) * h in ONE fused DVE op (unblocks the
            # hh matmuls one op earlier on the critical chain)
            rh = work.tile([128, UC, 1, BL], BF16, tag="rh")
            rh_i = nc.vector._custom_dve(
                CLIP01_MUL, out=rh[:, :, 0, :], in0=pr[:, :, 0, :], in1=h_in)
            z_last = None
            for k in range(UC):
                for m in range(4):
                    zi = nc.tensor.matmul(
                        pz[:, m, 0, :],
                        lhsT=wrz_sb[:, k, m * 128:(m + 1) * 128],
                        rhs=h_in[:, k, :],
                        start=False,
                        stop=(k == UC - 1 and m == 3),
                        skip_group_check=True,
                    )
                    if k == 0 and m == 0:
                        # same-engine ordering only (no semaphore): keep the
                        # whole z-burst AFTER the r-burst on the PE so clip_r
                        # fires at r-end, with z filling the clip_r/rh window
                        add_dep_helper(zi.ins, r_last.ins, sync=False,
                                       reason="z-burst after r-burst on PE")
                    z_last = zi
            # z-path off the critical chain, fused: w = 1 - clip(pz,0,1),
            # a = clip(pz,0,1) * h (z never materialized)
            w_t = work.tile([128, 4, 1, BL], BF16, tag="wt")
            w_i = nc.vector._custom_dve(
                GRU_W, out=w_t[:, :, 0, :], in0=pz[:, :, 0, :])
            add_dep_helper(w_i.ins, rh_i.ins, sync=False,
                           reason="DVE critical chain first")
            a_t = work.tile([128, 4, 1, BL], BF16, tag="at")
            nc.vector._custom_dve(
                CLIP01_MUL, out=a_t[:, :, 0, :], in0=pz[:, :, 0, :], in1=h_in)
            # hh pre-activation: psum = mx_h' + rh @ W_h; m-halves with the
            # tanh/blend for each half emitted right after its 8 MMs so each
            # half's chain starts as soon as its psum is complete
            hA_last = None
            for m in range(2):
                for k in range(UC):
                    hA_last = nc.tensor.matmul(
                        phA[:, m, 0, :],
                        lhsT=wrh_sb[:, k, m * 128:(m + 1) * 128],
                        rhs=rh[:, k, 0, :],
                        start=False,
                        stop=(m == 1 and k == UC - 1),
                        skip_group_check=True,
                    )
            # hh = tanh(psum); h' = (1-z)*hh + z*h -> hist out_slot (A half)
            hh_A = work.tile([128, 2, 1, BL], BF16, tag="hhA2")
            nc.scalar.activation(out=hh_A, in_=phA, func=Act.Tanh)
            f_A = work.tile([128, 2, 1, BL], BF16, tag="ftA")
            nc.vector.tensor_mul(f_A, w_t[:, 0:2, :, :], hh_A)
            nc.vector.tensor_add(hist[:, 0:2, out_slot:out_slot + 1, :],
                                 f_A, a_t[:, 0:2, :, :])
            for m in range(2, 4):
                for k in range(UC):
                    hB = nc.tensor.matmul(
                        phB[:, m - 2, 0, :],
                        lhsT=wrh_sb[:, k, m * 128:(m + 1) * 128],
                        rhs=rh[:, k, 0, :],
                        start=False,
                        stop=(m == 3 and k == UC - 1),
                        skip_group_check=True,
                    )

            hh_B = work.tile([128, 2, 1, BL], BF16, tag="hhB2")
            nc.scalar.activation(out=hh_B, in_=phB, func=Act.Tanh)
            f_B = work.tile([128, 2, 1, BL], BF16, tag="ftB")
            nc.vector.tensor_mul(f_B, w_t[:, 2:4, :, :], hh_B)
            nc.vector.tensor_add(hist[:, 2:4, out_slot:out_slot + 1, :],
                                 f_B, a_t[:, 2:4, :, :])
            return make_ids(*nxt) if nxt is not None else None

        with tc.For_i(0, T, BODY, staggered_reset=True,
                      hint_engines=(ET.PE, ET.DVE, ET.Activation,
                                    ET.SP, ET.Pool)) as i:
            nc.sync.dma_start(out=stgB,
                              in_=mx_sb[:, :, bass.ds(i + SBLK, SBLK), :])
            pre = None
            for s in range(SBLK):
                nxt = (stgA, s + 1) if s < SBLK - 1 else (stgB, 0)
                pre = step(stgA, s, s, pre, nxt)
            nc.sync.dma_start(out=stgA,
                              in_=mx_sb[:, :, bass.ds(i + BODY, SBLK), :])
            for s in range(SBLK):
                nxt = (stgB, s + 1) if s < SBLK - 1 else None
                pre = step(stgB, s, SBLK + s, pre, nxt)
            nc.sync.dma_start(out=out[:, :, bass.ds(i, BODY - 1), :],
                              in_=hist[:, :, 1:BODY, :])
            nc.sync.dma_start(out=out[:, :, bass.ds(i + BODY - 1, 1), :],
                              in_=hist[:, :, 0:1, :])
    return nc


def _graph():
    if "nc" not in _CACHE:
        nc = _build()
        if not nc.is_finalized():
            nc.finalize()
        _CACHE["nc"] = nc
    return _CACHE["nc"]


def kernel(x, kernel, recurrent_kernel, bias):
    global LAST_RESULT
    x = np.asarray(x, dtype=np.float32)
    wk_f = np.asarray(kernel, dtype=np.float32)
    wr_f = np.asarray(recurrent_kernel, dtype=np.float32)
    b_f = np.asarray(bias, dtype=np.float32)

    # fold hard_sigmoid affine (0.2*x + 0.5) into the z|r weight columns/bias
    scale = np.ones((3 * U,), np.float32)
    scale[: 2 * U] = 0.2
    wk_h = (wk_f * scale).astype(ml_dtypes.bfloat16)
    wr_s = wr_f * scale
    wrz_h = wr_s[:, :U].astype(ml_dtypes.bfloat16)
    wrr_h = wr_s[:, U:2 * U].astype(ml_dtypes.bfloat16)
    wrh_h = wr_s[:, 2 * U:].astype(ml_dtypes.bfloat16)
    bp_h = np.where(np.arange(3 * U) < 2 * U, 0.2 * b_f + 0.5, b_f).astype(np.float32)

    in_maps = []
    for c in range(NCORES):
        xs = x[c * BL:(c + 1) * BL]                       # [BL, T, D]
        xTc = np.ascontiguousarray(
            xs.transpose(2, 0, 1).reshape(D, BL * T)
        ).astype(ml_dtypes.bfloat16)
        in_maps.append({"xT": xTc, "wk": wk_h, "wrz": wrz_h,
                        "wrr": wrr_h, "wrh": wrh_h, "bp": bp_h})

    res = run_bass_kernel_spmd(
        _graph(), in_maps, core_ids=list(range(NCORES)),
        trace=bool(os.environ.get("GRU_TRACE")),
    )
    LAST_RESULT = res

    outs = []
    for c in range(NCORES):
        arr = np.asarray(res.results[c]["out"]).astype(np.float32)
        outs.append(np.transpose(arr, (3, 2, 1, 0)).reshape(BL, T, U))
    return np.concatenate(outs, axis=0)

